# revision 17
# baseline (speedup 1.0000x reference)
"""Trainium2 Bass kernel for nn_Attention_63934883168998.

Math (per token t): q,k,v = x W{q,k,v}^T reshaped (16 heads, 64); scores over
HEADS: S = q k^T / 8 (16x16), A = softmax(S), out = A v -> (1024); y = out Wo^T.

Sharding: pure data parallel over the 16384 tokens -> 2048 tokens/core.
All on-chip data fp16 (PE fp16 matmul = full rate).

Structure (v14): 256-token pairs; projections emitted ONE PAIR AHEAD of the
DVE score chain; the AV/output chain runs one pair behind.  Key changes vs
the v10 baseline:
 - The AV matmul is operand-swapped: v (from comb_k2) is the stationary, the
   block-diagonal A is the moving operand, split into even/odd-head halves
   whose PSUM outputs land at partitions 0-63 / 64-127 (col-tiling via
   base_partition).  The attention output therefore materializes ALREADY
   TRANSPOSED as [hid=(h%2)*64+d, token] slices that feed the Wo matmul
   directly: v10's 16 PE transposes, 16-DMA attn scatter, and oT copies are
   gone.  Heads are permuted parity-major host-side (h' = (h%2)*8 + h//2 for
   wq/wk/wv) so the A slots, the even/odd moving slices, and the standard
   hid order for Wo all line up with no extra data movement.
 - Startup: (wq_c, wk_c[:512], xT_c) triplet-ordered loads over 3 DMA-capable
   queues; the first tile's score product runs in two g-halves so only 3.5MB
   gates the first DVE op; wk's top half / wv / wot trail on sync+gpsimd.
 - Drain: pair NP-2's Wo projection is deferred into the last pair's
   scatter/abd DMA window (keeps the PE HAM-warm); the last pair's av halves
   use the freed projection PSUM banks; its y tiles ship per-T over 3 queues.
 - Block-diag zeroing on GpSimd, fast approx reciprocal for the softmax.
Measured: 403.2us (v10) -> ~370us; DVE busy ~306us is the bottleneck (the
score product+tree is elementwise on DVE at 2 elem/cycle, ~280us floor;
PE ~267us, ACT ~215us).
"""

import numpy as np
import ml_dtypes

N_CORES = 8
HID = 1024
NH, HD = 16, 64
TILE = 128
PAIR = 256
TPC = 16384 // N_CORES      # tokens per core
NP = TPC // PAIR            # tile-pairs per core (8)
NCH = HID // 128            # 8 hidden chunks
NG = TILE // 8              # 16 groups of 8 tokens

_cache = {}


def _build():
    if "nc" in _cache:
        return
    import concourse.bacc as bacc
    import concourse.mybir as mybir
    from concourse import tile

    f16 = mybir.dt.float16
    f32 = mybir.dt.float32
    AX = mybir.AxisListType
    OP = mybir.AluOpType
    AF = mybir.ActivationFunctionType

    nc = bacc.Bacc("TRN2", target_bir_lowering=False, debug=False)
    xt = nc.dram_tensor("xt", (HID, TPC), f16, kind="ExternalInput").ap()
    wts = {
        n: nc.dram_tensor(n, (HID, HID), f16, kind="ExternalInput").ap()
        for n in ("wqt", "wkt", "wvt", "wot")
    }
    y = nc.dram_tensor("y", (TPC, HID), f16, kind="ExternalOutput").ap()

    with tile.TileContext(nc) as tc:
        with (
            tc.tile_pool(name="wpool", bufs=1) as wpool,
            tc.tile_pool(name="io", bufs=2) as iop,
            tc.tile_pool(name="qk", bufs=2) as qkp,
            tc.tile_pool(name="sc", bufs=1) as scp,
            tc.tile_pool(name="av", bufs=2) as avp,
            tc.tile_pool(name="av1", bufs=1) as avp1,
            tc.tile_pool(name="st1", bufs=1) as st1,
            tc.tile_pool(name="psum", bufs=1, space="PSUM") as pp,
        ):
            # ---- resident weights / constants ----
            w_sb = {}
            for n in ("wqt", "wkt", "wvt", "wot"):
                w_sb[n] = wpool.tile([128, NCH, HID], f16, name=n + "_sb", tag=n)
            xt_r = xt.rearrange("(c p) t -> p c t", p=128)

            def load_xT(i):
                t = iop.tile([128, NCH, PAIR], f16, name=f"xT{i}", tag="xT")
                nc.sync.dma_start(t[:], xt_r[:, :, i * PAIR : (i + 1) * PAIR])
                return t

            # Critical-path loads in (wq_c, wk_c, xT_c) triplet order so chunk
            # c's projection matmuls fire as soon as its triplet lands; wv and
            # wot follow on sync+gpsimd ONLY -- the scalar queue must clear
            # early so the first q/k PSUM->SBUF copies aren't stuck behind
            # slot-limited DMA waits.
            xT0 = iop.tile([128, NCH, PAIR], f16, name="xT0", tag="xT")
            xT_bufs = {0: xT0}
            k = 0
            tengs = [nc.sync, nc.gpsimd, nc.sync, nc.scalar]
            # wk split by g-half: the first tile's score product runs in two
            # g-halves, so only wq + wk[:, 0:512] + xT0 (3.5MB, not 4.5MB)
            # gate the first DVE op; wk's top half arrives during the first
            # half's product.
            for c in range(NCH):
                tengs[k % 4].dma_start(
                    w_sb["wqt"][:, c, :], wts["wqt"][c * 128 : (c + 1) * 128, :]
                )
                k += 1
                tengs[k % 4].dma_start(
                    w_sb["wkt"][:, c, 0:512],
                    wts["wkt"][c * 128 : (c + 1) * 128, 0:512],
                )
                k += 1
                tengs[k % 4].dma_start(xT0[:, c, :], xt_r[:, c, 0:PAIR])
                k += 1
            for c in range(NCH):
                tengs[k % 4].dma_start(
                    w_sb["wkt"][:, c, 512:1024],
                    wts["wkt"][c * 128 : (c + 1) * 128, 512:1024],
                )
                k += 1
            for c in range(NCH):
                (nc.sync, nc.gpsimd)[c % 2].dma_start(
                    w_sb["wvt"][:, c, :], wts["wvt"][c * 128 : (c + 1) * 128, :]
                )
            for c in range(NCH):
                (nc.sync, nc.gpsimd)[c % 2].dma_start(
                    w_sb["wot"][:, c, :], wts["wot"][c * 128 : (c + 1) * 128, :]
                )
            # two static block-diag stationary buffers, zeroed once on GpSimd
            # (off the DVE queue); per pair only the 8 diagonal (b'=b) blocks
            # are rewritten via DMA.  A-slot order is (hp=h%2, hh=h//2).
            abd_bufs = []
            for j in range(2):
                ab = wpool.tile([128, NG, 8, NH, 2], f16, name=f"abds{j}",
                                tag=f"abds{j}")
                nc.gpsimd.memset(ab[:], 0.0)
                abd_bufs.append(ab)

            # ---- per-pair state kept across pipeline stages ----
            state = {}

            def new_state(i):
                state[i] = {
                    "xT": xT_bufs.pop(i),
                    "q": {}, "k": {},
                    "comb2": iop.tile([128, NH, 80, 2], f16, name=f"c2_{i}",
                                      tag="comb2"),
                    "comb_k2": avp.tile([128, NG, 80, 2], f16, name=f"ck2_{i}",
                                        tag="comb_k2"),
                }

            def proj(i, T, which):
                """Project tile (i,T). which in ('qk', 'v').  The very first
                tile emits k's top qdim-half last and copies k per-half, so
                the g<8 product can start before wk[:, 512:] arrives."""
                st = state[i]
                xT = st["xT"]
                first = (i == 0 and T == 0)
                if which == "qk":
                    ps_q = pp.tile([128, HID], f32, name=f"psq{i}_{T}", tag="psA")
                    ps_k = pp.tile([128, HID], f32, name=f"psk{i}_{T}", tag="psB")
                    order = [("wqt", ps_q, 0), ("wqt", ps_q, 1), ("wkt", ps_k, 0),
                             ("wkt", ps_k, 1)]
                    phases = ([order[:3], order[3:]] if first else [order])
                    for ph in phases:
                        for c in range(NCH):
                            stat = xT[:, c, T * TILE : (T + 1) * TILE]
                            for n, ps, h in ph:
                                nc.tensor.matmul(
                                    ps[:, h * 512 : (h + 1) * 512],
                                    stat,
                                    w_sb[n][:, c, h * 512 : (h + 1) * 512],
                                    start=(c == 0),
                                    stop=(c == NCH - 1),
                                )
                    q_sb = qkp.tile([128, NH, HD], f16, name=f"q{i}_{T}", tag=f"q{T}")
                    k_sb = qkp.tile([128, NH, HD], f16, name=f"k{i}_{T}", tag=f"k{T}")
                    nc.scalar.copy(q_sb[:].rearrange("p h d -> p (h d)"), ps_q[:])
                    if first:
                        for h in range(2):
                            nc.scalar.copy(
                                k_sb[:, h * 8 : (h + 1) * 8, :].rearrange(
                                    "p h d -> p (h d)"
                                ),
                                ps_k[:, h * 512 : (h + 1) * 512],
                            )
                    else:
                        nc.scalar.copy(k_sb[:].rearrange("p h d -> p (h d)"), ps_k[:])
                    st["q"][T] = q_sb
                    st["k"][T] = k_sb
                else:
                    ps_v = pp.tile([128, HID], f32, name=f"psv{i}_{T}", tag="psC")
                    for c in range(NCH):
                        stat = xT[:, c, T * TILE : (T + 1) * TILE]
                        for h in range(2):
                            nc.tensor.matmul(
                                ps_v[:, h * 512 : (h + 1) * 512],
                                stat,
                                w_sb["wvt"][:, c, h * 512 : (h + 1) * 512],
                                start=(c == 0),
                                stop=(c == NCH - 1),
                            )
                    # comb2[t, g, 0:16, T] = A (later); [t, g, 16:80, T] = v
                    nc.scalar.copy(
                        st["comb2"][:, :, 16:, T],
                        ps_v[:].rearrange("p (g d) -> p g d", g=NH),
                    )

            def prod_op(i, T):
                """DVE product for tile (i,T): prod[t, h, g, d] (h-major so the
                softmax g-sum reduces a contiguous inner axis).  First tile
                runs in two g-halves to start before wk's top half lands."""
                st = state[i]
                q_sb, k_sb = st["q"][T], st["k"][T]
                prod = st1.tile([128, NH, NH, HD], f16, name=f"pr{i}{T}", tag="prod")
                if i == 0 and T == 0:
                    for g in range(2):
                        q_ap = q_sb[:].unsqueeze(2).broadcast_to(
                            (128, NH, 8, HD)
                        )
                        k_ap = k_sb[:, g * 8 : (g + 1) * 8, :].unsqueeze(
                            1
                        ).broadcast_to((128, NH, 8, HD))
                        nc.vector.tensor_tensor(
                            prod[:, :, g * 8 : (g + 1) * 8, :], k_ap, q_ap,
                            op=OP.mult,
                        )
                else:
                    q_ap = q_sb[:].unsqueeze(2).broadcast_to((128, NH, NH, HD))
                    k_ap = k_sb[:].unsqueeze(1).broadcast_to((128, NH, NH, HD))
                    nc.vector.tensor_tensor(prod[:], k_ap, q_ap, op=OP.mult)
                st["prod"] = prod

            def tree(i, T):
                """DVE tree levels 1-4 over d; level-4 output parks in the
                pair-shared scrB2 so levels 5-6 run pair-merged."""
                st = state[i]
                p3 = st["prod"][:].rearrange("p h g d -> p (h g) d")
                scrA = st1.tile([128, NH * NH, 32], f16, tag="scrA")
                scrB = st1.tile([128, NH * NH, 16], f16, tag="scrB")
                if T == 0:
                    st["scrB2"] = scp.tile([128, 2, NH * NH, 4], f16,
                                           name=f"sb2_{i}", tag="scrB2")
                with nc.allow_low_precision(reason="fp16 score partials"):
                    nc.vector.tensor_tensor(
                        scrA[:], p3[:, :, 0:32], p3[:, :, 32:64], op=OP.add
                    )
                    nc.vector.tensor_tensor(
                        scrB[:], scrA[:, :, 0:16], scrA[:, :, 16:32], op=OP.add
                    )
                    nc.vector.tensor_tensor(
                        scrA[:, :, 0:8], scrB[:, :, 0:8], scrB[:, :, 8:16], op=OP.add
                    )
                    nc.vector.tensor_tensor(
                        st["scrB2"][:, T, :, :], scrA[:, :, 0:4], scrA[:, :, 4:8],
                        op=OP.add,
                    )

            def tail5(i):
                """Pair-merged tree levels 5-6 (both tiles in one op each)."""
                st = state[i]
                scrB2 = st["scrB2"]
                scrA5 = scp.tile([128, 2, NH * NH, 2], f16, tag="scrA5")
                scores2 = scp.tile([128, 2, NH * NH], f16, name=f"s{i}",
                                   tag="scores2")
                with nc.allow_low_precision(reason="fp16 score partials"):
                    nc.vector.tensor_tensor(
                        scrA5[:], scrB2[:, :, :, 0:2], scrB2[:, :, :, 2:4],
                        op=OP.add,
                    )
                    nc.vector.tensor_tensor(
                        scores2[:].unsqueeze(3),
                        scrA5[:, :, :, 0:1],
                        scrA5[:, :, :, 1:2],
                        op=OP.add,
                    )
                st["scores2"] = scores2

            def exp_op(i):
                st = state[i]
                ex2 = scp.tile([128, 2, NH * NH], f16, name=f"ex{i}", tag="ex2")
                nc.scalar.activation(ex2[:], st["scores2"][:], AF.Exp, scale=0.125)
                st["ex2"] = ex2

            def softmax_tail(i):
                """Pair-merged DVE ssum-reduce + fast recip + A-normalize,
                positioned after the next product so nothing head-blocks.
                A is written into comb2 slots in (hp, hh) head order."""
                st = state[i]
                ex2 = st["ex2"]
                ssum2 = scp.tile([128, 2, NH], f32, tag="ssum2")
                ex_hg = ex2[:].rearrange("p T (h g) -> p T h g", h=NH)
                nc.vector.tensor_reduce(ssum2[:], ex_hg, axis=AX.X, op=OP.add)
                rs2 = scp.tile([128, 2, NH], f32, tag="rs2")
                nc.vector.reciprocal_approx_fast(
                    rs2[:].rearrange("p T h -> p (T h)"),
                    ssum2[:].rearrange("p T h -> p (T h)"),
                )
                # A into comb2 A-slots (both tiles).  Heads are already in
                # parity-major order h' = (h%2)*8 + h//2 (host-side wq/wk/wv
                # row permutation), so slot s = h' directly:
                # comb2[t, g, h', T] = ex2[t, T, (h',g)] * rs2[t, T, h']
                nc.vector.tensor_tensor(
                    st["comb2"][:, :, 0:16, :],
                    ex2[:].rearrange("p T (h g) -> p g h T", h=NH),
                    rs2[:]
                    .rearrange("p T h -> p h T")
                    .unsqueeze(1)
                    .broadcast_to((128, NH, NH, 2)),
                    op=OP.mult,
                )

            def comb_scatter(i):
                """comb2 -> comb_k2: (b,g)-partition layout, both tiles per DMA
                (320B contiguous runs)."""
                st = state[i]
                comb2, comb_k2 = st["comb2"], st["comb_k2"]
                engs = (nc.sync, nc.scalar, nc.gpsimd) \
                    if i == NP - 1 else (nc.sync, nc.scalar)
                for grp in range(NG):
                    engs[grp % len(engs)].dma_start(
                        comb_k2[:, grp, :, :],
                        comb2[grp * 8 : (grp + 1) * 8, :, :, :],
                    )

            def abd(i):
                """Overwrite the 8 diagonal blocks of the static block-diag
                stationary with this pair's A values (sbuf->sbuf DMAs; engines
                can't start at partition offset 16)."""
                st = state[i]
                ab = abd_bufs[i % 2]
                engs = (nc.sync, nc.scalar, nc.gpsimd) \
                    if i == NP - 1 else (nc.sync, nc.scalar)
                for b in range(8):
                    engs[b % len(engs)].dma_start(
                        ab[b * 16 : (b + 1) * 16, :, b, :, :],
                        st["comb_k2"][b * 16 : (b + 1) * 16, :, 0:16, :],
                    )
                st["abd2"] = ab

            # in the drain the projection PSUM banks are free: the last
            # pair's av halves get their own banks (no copy-wait serializing)
            AV_DRAIN_TAGS = {(0, 0): "psA", (1, 0): "psC",
                             (0, 1): "psB", (1, 1): "pav1"}

            def av_h(e, T, half):
                """AV matmuls for tile (e,T), token-groups half*8..half*8+8,
                operand-swapped: stationary = v, moving = block-diag A
                (even/odd-head slices), PSUM partitions (h%2)*64+d.  The
                result is the attention output already transposed to
                [hid, token] form."""
                st = state[e]
                abd2, comb_k2 = st["abd2"], st["comb_k2"]
                if T == 0 and half == 0:
                    st["oT"] = avp1.tile(
                        [128, 2, NG, 8, 8], f16, name=f"oT{e}", tag="oT_all"
                    )
                tag = AV_DRAIN_TAGS[(T, half)] if e == NP - 1 else f"pav{T}"
                pv = pp.tile([128, 8, 8, 8], f32, name=f"pv{e}{T}{half}",
                             tag=tag)
                for g8 in range(8):
                    grp = half * 8 + g8
                    for hp in range(2):
                        nc.tensor.matmul(
                            pv[hp * 64 : (hp + 1) * 64, g8, :, :],
                            comb_k2[:, grp, 16:, T],
                            abd2[:, grp, :, hp * 8 : (hp + 1) * 8, T],
                            start=True,
                            stop=True,
                        )
                nc.scalar.copy(
                    st["oT"][:, T, half * 8 : (half + 1) * 8, :, :], pv[:]
                )

            def wo2(e):
                """Output projection for BOTH tiles straight from the
                transposed attention output: lhsT = oT[:, T, :, :, c] is the
                hid-chunk-c stationary, moving = wot chunk."""
                st = state[e]
                st["y_sb2"] = avp1.tile(
                    [128, 2, HID], f16, name=f"ysb{e}", tag="y_sb2"
                )
                for T in range(2):
                    ptags = ((("pav0", "psA"), ("psB", "psC"))[T]
                             if e == NP - 1 else ("pav0", "pav1"))
                    py = [
                        pp.tile([128, 512], f32, name=f"py{e}{T}{h}",
                                tag=ptags[h])
                        for h in range(2)
                    ]
                    for c in range(NCH):
                        for h in range(2):
                            nc.tensor.matmul(
                                py[h][:],
                                st["oT"][:, T, :, :, c],
                                w_sb["wot"][:, c, h * 512 : (h + 1) * 512],
                                start=(c == 0),
                                stop=(c == NCH - 1),
                            )
                    for h in range(2):
                        nc.scalar.copy(
                            st["y_sb2"][:, T, h * 512 : (h + 1) * 512], py[h][:]
                        )
                    if e == NP - 1:
                        # drain-critical: ship tile T while T'=1 still computes
                        t0 = e * PAIR
                        ydst = y[t0 : t0 + PAIR, :].rearrange(
                            "(T2 t) f -> t T2 f", T2=2
                        )
                        engs = ((nc.sync, nc.scalar), (nc.gpsimd, nc.sync))[T]
                        for j in range(2):
                            engs[j].dma_start(
                                ydst[:, T, j * 512 : (j + 1) * 512],
                                st["y_sb2"][:, T, j * 512 : (j + 1) * 512],
                            )

            def y_out(e):
                if e == NP - 1:
                    return  # shipped per-tile inside wo2
                st = state[e]
                t0 = e * PAIR
                ydst = y[t0 : t0 + PAIR, :].rearrange("(T t) f -> t T f", T=2)
                nc.sync.dma_start(ydst, st["y_sb2"][:])

            xT_bufs[1] = load_xT(1)

            # ---- main loop: projections ONE pair ahead, AV one pair behind --
            # iteration i: proj(i+1), DVE scores(i), AV/output chain(i-1)
            new_state(0)
            proj(0, 0, "qk")
            proj(0, 0, "v")
            proj(0, 1, "qk")
            proj(0, 1, "v")
            for i in range(NP + 1):
                e = i - 1
                if i + 1 < NP:
                    new_state(i + 1)
                    proj(i + 1, 0, "qk")
                if i < NP:
                    prod_op(i, 0)
                if e >= 0:
                    softmax_tail(e)     # deferred: sits after prod(i,T0)
                    comb_scatter(e)
                    abd(e)
                if i == NP:
                    # pair NP-2's output projection was held back so its PE
                    # work fills the last pair's scatter/abd DMA window
                    # (keeps the PE HAM-warm through the drain)
                    wo2(NP - 2)
                    y_out(NP - 2)
                if i + 1 < NP:
                    proj(i + 1, 0, "v")
                    proj(i + 1, 1, "qk")
                if i < NP:
                    tree(i, 0)
                if e >= 0:
                    av_h(e, 0, 0)
                    av_h(e, 1, 0)
                    av_h(e, 0, 1)
                    av_h(e, 1, 1)
                if i < NP:
                    prod_op(i, 1)
                if i + 1 < NP:
                    proj(i + 1, 1, "v")
                if i < NP:
                    tree(i, 1)
                    tail5(i)
                    exp_op(i)   # ACT: ahead of y copies so the deferred
                                # reduce never waits on the exp
                if e >= 0 and e != NP - 2:
                    wo2(e)
                    y_out(e)
                    if e == NP - 1:
                        del state[NP - 2]
                    del state[e]
                if i + 2 < NP:
                    xT_bufs[i + 2] = load_xT(i + 2)

    nc.compile()
    _cache["nc"] = nc


def _prep_inputs(x, wq, wk, wv, wo):
    x2 = np.asarray(x, dtype=np.float32).reshape(-1, HID)
    # Heads reordered parity-major (h' = hp*8+hh <-> original 2*hh+hp) for
    # wq/wk/wv so the AV moving slices are contiguous and the transposed
    # attention output maps to the standard hid order (wo unpermuted).
    perm = [2 * hh + hp for hp in (0, 1) for hh in range(8)]

    def headperm(w):
        w32 = np.asarray(w, dtype=np.float32)
        return w32.reshape(NH, HD, HID)[perm].reshape(HID, HID)

    w16 = {
        n: np.ascontiguousarray(w.T).astype(np.float16)
        for n, w in (
            ("wqt", headperm(wq)),
            ("wkt", headperm(wk)),
            ("wvt", headperm(wv)),
            ("wot", np.asarray(wo, dtype=np.float32)),
        )
    }
    in_maps = []
    for i in range(N_CORES):
        sh = x2[i * TPC : (i + 1) * TPC].astype(np.float16)
        m = {"xt": np.ascontiguousarray(sh.T)}
        m.update(w16)
        in_maps.append(m)
    return in_maps


def kernel(x, wq, wk, wv, wo, _trace=False):
    from concourse import bass_utils

    _build()
    in_maps = _prep_inputs(x, wq, wk, wv, wo)
    res = bass_utils.run_bass_kernel_spmd(
        _cache["nc"], in_maps, core_ids=list(range(N_CORES)), trace=_trace
    )
    kernel.last_result = res
    B, S = 4, 4096
    out = np.concatenate([r["y"] for r in res.results], axis=0)
    return out.reshape(B, S, HID).astype(np.float32)
